# revision 8
# baseline (speedup 1.0000x reference)
"""GNN message-passing layer on 8 TRN2 NeuronCores.

Math: y[e] = relu(concat(x[i[e]], x[i[e]]) @ W1 + b1) @ W2 + b2
         = relu(x[i[e]] @ (W1[:C]+W1[C:]) + b1) @ W2 + b2.
The MLP depends only on the source node, so compute z = MLP(x) once per
node (50k rows), then y = z[nbr_idx] is a pure gather (800k rows).

Sharding: edges are split evenly across the 8 cores; each core computes
the full z table locally (x + weights replicated; phase A is tiny) and
then gathers + writes its own edge shard. No collectives.

Phase B uses the GPSIMD dma_gather custom instruction. Its indices are
signed int16, so the bf16 z table is gathered at pair-row granularity
(row = 2 nodes = 512B, pair id < 25088 fits int16); a DVE predicated
copy then selects the right half per edge (mask = idx & 1) and upcasts
to f32. Edge->position packing is chosen so the per-tile y write is one
contiguous 8KB descriptor per partition.
"""

from contextlib import ExitStack

import ml_dtypes
import numpy as np

import concourse.bacc as bacc
import concourse.mybir as mybir
import concourse.tile as tile
from concourse import library_config
from concourse.bass_utils import run_bass_kernel_spmd
from concourse.masks import make_identity

N_CORES = 8
C = 128  # channels (C_IN == C_OUT)
N_NODES = 50000
E_TOTAL = 800000

ACH = 512  # phase-A node chunk (max fp32 moving dim)
NPAD = ((N_NODES + ACH - 1) // ACH) * ACH  # 50176
NCH = NPAD // ACH  # 98

EPC = E_TOTAL // N_CORES  # 100000 edges per core
NI = 2048  # edges per dma_gather tile
TBB = (EPC + NI - 1) // NI  # 49 gather tiles
EPC_PAD = TBB * NI  # 100352
KCH = NI // 128  # 16 gathered chunks per partition

F32 = mybir.dt.float32
BF16 = mybir.dt.bfloat16

# matmul input dtype for phase A (float32 / float32r tradeoff)
MM_DT = mybir.dt.float32


import os
PHASES = os.environ.get("KPHASES", "AB")


def _build_nc():
    nc = bacc.Bacc("TRN2", target_bir_lowering=False, debug=False,
                   num_devices=N_CORES, dynamic_dma_scratch_size=131072)

    xT = nc.dram_tensor("xT", [C, NPAD], F32, kind="ExternalInput")
    idx16 = nc.dram_tensor("idx16", [128, EPC_PAD // 16], mybir.dt.int16,
                           kind="ExternalInput")
    parity = nc.dram_tensor("parity", [128, EPC_PAD // 128], mybir.dt.uint8,
                            kind="ExternalInput")
    w1 = nc.dram_tensor("w1", [C, C], F32, kind="ExternalInput")
    w2 = nc.dram_tensor("w2", [C, C], F32, kind="ExternalInput")
    b1 = nc.dram_tensor("b1", [C, 1], F32, kind="ExternalInput")
    b2 = nc.dram_tensor("b2", [C, 1], F32, kind="ExternalInput")
    y = nc.dram_tensor("y", [EPC_PAD, C], F32, kind="ExternalOutput")
    zkind = "ExternalOutput" if PHASES == "A" else \
        ("ExternalInput" if PHASES == "B" else "Internal")
    z = nc.dram_tensor("z_table", [NPAD, C], BF16, kind=zkind)

    with tile.TileContext(nc) as tc, ExitStack() as ctx:
        const = ctx.enter_context(tc.tile_pool(name="const", bufs=1))
        xpool = ctx.enter_context(tc.tile_pool(name="xin", bufs=3))
        hpool = ctx.enter_context(tc.tile_pool(name="hbuf", bufs=3))
        zb_pool = ctx.enter_context(tc.tile_pool(name="zb", bufs=3))
        gpool = ctx.enter_context(tc.tile_pool(name="gbuf", bufs=3))
        spool = ctx.enter_context(tc.tile_pool(name="sel", bufs=3))
        psA = ctx.enter_context(tc.tile_pool(name="psA", bufs=2, space="PSUM"))
        psT = ctx.enter_context(tc.tile_pool(name="psT", bufs=4, space="PSUM"))

        w1t = const.tile([C, C], F32)
        w2t = const.tile([C, C], F32)
        b1t = const.tile([C, 1], F32)
        b2t = const.tile([C, 1], F32)
        ident = const.tile([128, 128], F32)
        idxt = const.tile([128, EPC_PAD // 16], mybir.dt.int16)
        maskt = const.tile([128, EPC_PAD // 128], mybir.dt.uint8)
        nc.sync.dma_start(out=w1t[:], in_=w1[:])
        nc.sync.dma_start(out=w2t[:], in_=w2[:])
        nc.sync.dma_start(out=b1t[:], in_=b1[:])
        nc.sync.dma_start(out=b2t[:], in_=b2[:])
        nc.sync.dma_start(out=idxt[:], in_=idx16[:])
        nc.sync.dma_start(out=maskt[:], in_=parity[:])
        make_identity(nc, ident[:])

        # ---- Phase A: z[n]  (skipped when PHASES=="B") = relu(x[n] @ W1eff + b1) @ W2 + b2, computed in
        # transposed orientation per 512-node chunk, PE-transposed back in
        # 4-interleaved column groups so each chunk is one contiguous write.
        for t in range(NCH if "A" in PHASES else 0):
            xt = xpool.tile([C, ACH], F32)
            nc.sync.dma_start(out=xt[:], in_=xT[:, t * ACH:(t + 1) * ACH])

            h_ps = psA.tile([C, ACH], F32, tag="h_ps")
            nc.tensor.matmul(h_ps[:], w1t[:].bitcast(MM_DT),
                             xt[:].bitcast(MM_DT), start=True, stop=True)
            h_sb = hpool.tile([C, ACH], F32, tag="h_sb")
            nc.scalar.activation(h_sb[:], h_ps[:],
                                 mybir.ActivationFunctionType.Relu,
                                 bias=b1t[:, 0:1])

            z_ps = psA.tile([C, ACH], F32, tag="z_ps")
            nc.tensor.matmul(z_ps[:], w2t[:].bitcast(MM_DT),
                             h_sb[:].bitcast(MM_DT), start=True, stop=True)
            zt_sb = hpool.tile([C, ACH], F32, tag="zt_sb")
            nc.scalar.activation(zt_sb[:], z_ps[:],
                                 mybir.ActivationFunctionType.Identity,
                                 bias=b2t[:, 0:1])

            # transpose column group j (cols j, j+4, ...) -> rows n0+4q+j
            zbuf = zb_pool.tile([128, ACH // 128, C], BF16, tag="zbuf")
            for j in range(ACH // 128):
                tr_ps = psT.tile([128, 128], F32, tag="tr")
                nc.tensor.transpose(tr_ps[:], zt_sb[:, j:ACH:4], ident[:])
                nc.vector.tensor_copy(zbuf[:, j, :], tr_ps[:])
            n0 = t * ACH
            nc.sync.dma_start(
                out=z[n0:n0 + ACH, :].rearrange("(q j) c -> q j c", j=4),
                in_=zbuf[:])

        tc.strict_bb_all_engine_barrier()

        # ---- Phase B: dma_gather pair-rows + DVE half-select, write y shard.
        nc.gpsimd.load_library(library_config.mlp)
        zview = z[:].rearrange("(a two) c -> a (two c)", two=2)  # [NPAD/2,2C]
        for t in range(TBB if "B" in PHASES else 0):
            g = gpool.tile([128, KCH, 2 * C], BF16, tag="g")
            nc.gpsimd.dma_gather(
                out_ap=g[:], in_ap=zview,
                idxs_ap=idxt[:, t * (NI // 16):(t + 1) * (NI // 16)],
                num_idxs=NI, num_idxs_reg=NI, elem_size=2 * C,
                single_packet=False)
            even = g[:, :, 0:C]
            odd = g[:, :, C:2 * C]
            m = maskt[:, t * KCH:(t + 1) * KCH].to_broadcast([128, KCH, C])
            nc.vector.copy_predicated(out=even, mask=m, data=odd)
            sel = spool.tile([128, KCH, C], F32, tag="sel")
            nc.vector.tensor_copy(sel[:], even)
            # position (p, tl) holds edge row p*KCH + tl of this tile
            nc.sync.dma_start(
                out=y[t * NI:(t + 1) * NI, :].rearrange(
                    "(p tl) c -> p tl c", tl=KCH),
                in_=sel[:])

    nc.compile()
    return nc


_NC_CACHE = None


def _get_nc():
    global _NC_CACHE
    if _NC_CACHE is None:
        _NC_CACHE = _build_nc()
    return _NC_CACHE


def _pack_indices(idx_pad):
    """idx_pad: int32 [EPC_PAD] -> (idx16 [128, EPC_PAD//16] int16,
    parity [128, EPC_PAD//128] bf16) in the position layout where edge row
    r (within a tile) sits at gather position i = (r%16)*128 + r//16."""
    pair = (idx_pad >> 1).astype(np.int16)
    par = (idx_pad & 1).astype(np.uint8)

    r = np.arange(NI)
    pos = (r % 16) * 128 + r // 16  # position of row r

    pair_t = pair.reshape(TBB, NI)
    pair_by_pos = np.empty((TBB, NI), dtype=np.int16)
    pair_by_pos[:, pos] = pair_t
    # wrap: position i at [i%16, i//16] per tile, tiles side by side
    idx16 = (pair_by_pos.reshape(TBB, NI // 16, 16)
             .transpose(2, 0, 1).reshape(16, TBB * (NI // 16)))
    idx16 = np.tile(np.ascontiguousarray(idx16), (8, 1))

    # mask[p, t*KCH + tl] = parity of edge row p*KCH + tl of tile t
    mask = (par.reshape(TBB, 128, KCH).transpose(1, 0, 2)
            .reshape(128, TBB * KCH))
    return idx16, np.ascontiguousarray(mask).astype(np.uint8)


def kernel(x, nbr_idx, W1, b1, W2, b2, _trace=False, _trace_kwargs=None):
    x = np.asarray(x, dtype=np.float32)
    nbr_idx_np = np.asarray(nbr_idx)
    W1 = np.asarray(W1, dtype=np.float32)
    W2 = np.asarray(W2, dtype=np.float32)
    b1 = np.asarray(b1, dtype=np.float32)
    b2 = np.asarray(b2, dtype=np.float32)

    w1eff = np.ascontiguousarray(W1[:C] + W1[C:])  # [C, C]
    xT = np.zeros((C, NPAD), dtype=np.float32)
    xT[:, :N_NODES] = x.T

    in_maps = []
    for i in range(N_CORES):
        idx_pad = np.zeros(EPC_PAD, dtype=np.int32)
        idx_pad[:EPC] = nbr_idx_np[i * EPC:(i + 1) * EPC].astype(np.int32)
        idx16, mask = _pack_indices(idx_pad)
        in_maps.append({
            "xT": xT,
            "idx16": idx16,
            "parity": mask,
            "w1": w1eff,
            "w2": W2,
            "b1": b1.reshape(C, 1),
            "b2": b2.reshape(C, 1),
        })

    nc = _get_nc()
    res = run_bass_kernel_spmd(nc, in_maps, list(range(N_CORES)),
                               trace=_trace, **(_trace_kwargs or {}))

    out = np.empty((E_TOTAL, C), dtype=np.float32)
    for i in range(N_CORES):
        out[i * EPC:(i + 1) * EPC] = res.results[i]["y"][:EPC]
    if _trace:
        return out, res
    return out


# revision 19
# speedup vs baseline: 41988.8370x; 41988.8370x over previous
"""GNN message-passing layer on 8 TRN2 NeuronCores.

Math: y[e] = relu(concat(x[i[e]], x[i[e]]) @ W1 + b1) @ W2 + b2
         = relu(x[i[e]] @ (W1[:C]+W1[C:]) + b1) @ W2 + b2.
The MLP depends only on the source node, so compute z = MLP(x) once per
node (50k rows), then y = z[nbr_idx] is a pure gather (800k rows).

Sharding: edges are split evenly across the 8 cores; each core computes
the full z table locally (x + weights replicated; phase A is tiny) and
then gathers + writes its own edge shard. No collectives.

Phase B uses the GPSIMD dma_gather custom instruction. Its indices are
signed int16, so the bf16 z table is gathered at pair-row granularity
(row = 2 nodes = 512B, pair id < 25088 fits int16); a DVE predicated
copy then selects the right half per edge (mask = idx & 1) and upcasts
to f32. Edge->position packing is chosen so the per-tile y write is one
contiguous 8KB descriptor per partition.
"""

from contextlib import ExitStack

import ml_dtypes
import numpy as np

import concourse.bacc as bacc
import concourse.mybir as mybir
import concourse.tile as tile
from concourse import library_config
from concourse.bass_utils import run_bass_kernel_spmd
from concourse.masks import make_identity

N_CORES = 8
C = 128  # channels (C_IN == C_OUT)
N_NODES = 50000
E_TOTAL = 800000

ACH = 512  # phase-A compute chunk (max moving dim per matmul)
SCH = 2048  # phase-A DMA super-chunk (one x load + one z write)
NPAD = ((N_NODES + SCH - 1) // SCH) * SCH  # 51200
NCH = NPAD // ACH  # 100

EPC = E_TOTAL // N_CORES  # 100000 edges per core
NI = 2048  # edges per dma_gather tile
TBB = (EPC + NI - 1) // NI  # 49 gather tiles
EPC_PAD = TBB * NI  # 100352
KCH = NI // 128  # 16 gathered chunks per partition

F32 = mybir.dt.float32
BF16 = mybir.dt.bfloat16

# matmul input dtype for phase A
MM_DT = mybir.dt.bfloat16


import os
PHASES = os.environ.get("KPHASES", "AB")


def _build_nc():
    nc = bacc.Bacc("TRN2", target_bir_lowering=False, debug=False,
                   num_devices=N_CORES, dynamic_dma_scratch_size=131072)

    xT = nc.dram_tensor("xT", [C, NPAD], BF16, kind="ExternalInput")
    idx16 = nc.dram_tensor("idx16", [128, EPC_PAD // 16], mybir.dt.int16,
                           kind="ExternalInput")
    parity = nc.dram_tensor("parity", [128, EPC_PAD // 128], mybir.dt.uint8,
                            kind="ExternalInput")
    w1 = nc.dram_tensor("w1", [C, C], BF16, kind="ExternalInput")
    w2 = nc.dram_tensor("w2", [C, C], BF16, kind="ExternalInput")
    b1 = nc.dram_tensor("b1", [C, 1], F32, kind="ExternalInput")
    b2 = nc.dram_tensor("b2", [C, 1], F32, kind="ExternalInput")
    y = nc.dram_tensor("y", [EPC_PAD, C], F32, kind="ExternalOutput")
    zkind = "ExternalOutput" if PHASES == "A" else \
        ("ExternalInput" if PHASES == "B" else "Internal")
    z = nc.dram_tensor("z_table", [NPAD, C], BF16, kind=zkind)

    with tile.TileContext(nc) as tc, ExitStack() as ctx:
        const = ctx.enter_context(tc.tile_pool(name="const", bufs=1))
        xpool = ctx.enter_context(tc.tile_pool(name="xin", bufs=3))
        hpool = ctx.enter_context(tc.tile_pool(name="hbuf", bufs=3))
        zb_pool = ctx.enter_context(tc.tile_pool(name="zb", bufs=3))
        gpool = ctx.enter_context(tc.tile_pool(name="gbuf", bufs=3))
        spool = ctx.enter_context(tc.tile_pool(name="sel", bufs=3))
        psA = ctx.enter_context(tc.tile_pool(name="psA", bufs=2, space="PSUM"))
        psT = ctx.enter_context(tc.tile_pool(name="psT", bufs=2, space="PSUM"))

        w1t = const.tile([C, C], MM_DT)
        w2t = const.tile([C, C], MM_DT)
        b1t = const.tile([C, 1], F32)
        b2t = const.tile([C, 1], F32)
        ident = const.tile([128, 128], BF16)
        idxt = const.tile([128, EPC_PAD // 16], mybir.dt.int16)
        maskt = const.tile([128, EPC_PAD // 128], mybir.dt.uint8)
        nc.sync.dma_start(out=w1t[:], in_=w1[:])
        nc.sync.dma_start(out=w2t[:], in_=w2[:])
        nc.sync.dma_start(out=b1t[:], in_=b1[:])
        nc.sync.dma_start(out=b2t[:], in_=b2[:])
        nc.sync.dma_start(out=idxt[:], in_=idx16[:])
        nc.sync.dma_start(out=maskt[:], in_=parity[:])
        make_identity(nc, ident[:])

        # ---- Phase A (skipped when PHASES=="B"):
        # z[n] = relu(x[n] @ W1eff + b1) @ W2 + b2 per 512-node chunk in
        # transposed orientation, PE-transposed back in 4-interleaved column
        # groups. DMA granularity is a 2048-node super-chunk: one x load and
        # one z write each.
        SUB = SCH // ACH
        for ts in range(NPAD // SCH if "A" in PHASES else 0):
            xt = xpool.tile([C, SCH], MM_DT)
            nc.sync.dma_start(out=xt[:], in_=xT[:, ts * SCH:(ts + 1) * SCH])
            # zbuf[q, b, j, c] = z[ts*SCH + 512b + 4q + j, c]
            zbuf = zb_pool.tile([128, SUB, ACH // 128, C], BF16, tag="zbuf")
            for b in range(SUB):
                h_ps = psA.tile([C, ACH], F32, tag="h_ps")
                nc.tensor.matmul(h_ps[:], w1t[:], xt[:, b * ACH:(b + 1) * ACH],
                                 start=True, stop=True)
                h_sb = hpool.tile([C, ACH], MM_DT, tag="h_sb")
                nc.scalar.activation(h_sb[:], h_ps[:],
                                     mybir.ActivationFunctionType.Relu,
                                     bias=b1t[:, 0:1])

                z_ps = psA.tile([C, ACH], F32, tag="z_ps")
                nc.tensor.matmul(z_ps[:], w2t[:], h_sb[:], start=True,
                                 stop=True)
                zt_sb = hpool.tile([C, ACH], BF16, tag="zt_sb")
                nc.vector.tensor_tensor(out=zt_sb[:], in0=z_ps[:],
                                        in1=b2t[:, 0:1].to_broadcast([C, ACH]),
                                        op=mybir.AluOpType.add)

                # transpose col group j (cols j, j+4, ...) -> rows +4q+j
                tr_ps = psT.tile([128, ACH // 128, 128], BF16, tag="tr")
                for j in range(ACH // 128):
                    nc.tensor.transpose(tr_ps[:, j, :], zt_sb[:, j:ACH:4],
                                        ident[:])
                if b % 2 == 0:
                    nc.vector.tensor_copy(zbuf[:, b, :, :], tr_ps[:])
                else:
                    nc.scalar.copy(zbuf[:, b, :, :], tr_ps[:])
            n0 = ts * SCH
            nc.scalar.dma_start(
                out=z[n0:n0 + SCH, :].rearrange("(b q j) c -> q b j c",
                                                b=SUB, j=4),
                in_=zbuf[:])

        tc.strict_bb_all_engine_barrier()

        # ---- Phase B: dma_gather pair-rows + DVE half-select, write y shard.
        nc.gpsimd.load_library(library_config.mlp)
        zview = z[:].rearrange("(a two) c -> a (two c)", two=2)  # [NPAD/2,2C]
        for t in range(TBB if "B" in PHASES else 0):
            g = gpool.tile([128, KCH, 2 * C], BF16, tag="g")
            nc.gpsimd.dma_gather(
                out_ap=g[:], in_ap=zview,
                idxs_ap=idxt[:, t * (NI // 16):(t + 1) * (NI // 16)],
                num_idxs=NI, num_idxs_reg=NI, elem_size=2 * C,
                single_packet=False)
            even = g[:, :, 0:C]
            odd = g[:, :, C:2 * C]
            m = maskt[:, t * KCH:(t + 1) * KCH].to_broadcast([128, KCH, C])
            nc.vector.copy_predicated(out=even, mask=m, data=odd)
            sel = spool.tile([128, KCH, C], F32, tag="sel")
            nc.scalar.copy(sel[:], even)
            # position (p, tl) holds edge row p*KCH + tl of this tile
            nc.sync.dma_start(
                out=y[t * NI:(t + 1) * NI, :].rearrange(
                    "(p tl) c -> p tl c", tl=KCH),
                in_=sel[:])

    nc.compile()
    return nc


_NC_CACHE = None


def _get_nc():
    global _NC_CACHE
    if _NC_CACHE is None:
        _NC_CACHE = _build_nc()
    return _NC_CACHE


def _pack_indices(idx_pad):
    """idx_pad: int32 [EPC_PAD] -> (idx16 [128, EPC_PAD//16] int16,
    parity [128, EPC_PAD//128] bf16) in the position layout where edge row
    r (within a tile) sits at gather position i = (r%16)*128 + r//16."""
    pair = (idx_pad >> 1).astype(np.int16)
    par = (idx_pad & 1).astype(np.uint8)

    r = np.arange(NI)
    pos = (r % 16) * 128 + r // 16  # position of row r

    pair_t = pair.reshape(TBB, NI)
    pair_by_pos = np.empty((TBB, NI), dtype=np.int16)
    pair_by_pos[:, pos] = pair_t
    # wrap: position i at [i%16, i//16] per tile, tiles side by side
    idx16 = (pair_by_pos.reshape(TBB, NI // 16, 16)
             .transpose(2, 0, 1).reshape(16, TBB * (NI // 16)))
    idx16 = np.tile(np.ascontiguousarray(idx16), (8, 1))

    # mask[p, t*KCH + tl] = parity of edge row p*KCH + tl of tile t
    mask = (par.reshape(TBB, 128, KCH).transpose(1, 0, 2)
            .reshape(128, TBB * KCH))
    return idx16, np.ascontiguousarray(mask).astype(np.uint8)


def kernel(x, nbr_idx, W1, b1, W2, b2, _trace=False, _trace_kwargs=None):
    x = np.asarray(x, dtype=np.float32)
    nbr_idx_np = np.asarray(nbr_idx)
    W1 = np.asarray(W1, dtype=np.float32)
    W2 = np.asarray(W2, dtype=np.float32)
    b1 = np.asarray(b1, dtype=np.float32)
    b2 = np.asarray(b2, dtype=np.float32)

    w1eff = np.ascontiguousarray(W1[:C] + W1[C:]).astype(ml_dtypes.bfloat16)
    w2_bf = W2.astype(ml_dtypes.bfloat16)
    xT = np.zeros((C, NPAD), dtype=ml_dtypes.bfloat16)
    xT[:, :N_NODES] = x.T.astype(ml_dtypes.bfloat16)

    in_maps = []
    for i in range(N_CORES):
        idx_pad = np.zeros(EPC_PAD, dtype=np.int32)
        idx_pad[:EPC] = nbr_idx_np[i * EPC:(i + 1) * EPC].astype(np.int32)
        idx16, mask = _pack_indices(idx_pad)
        in_maps.append({
            "xT": xT,
            "idx16": idx16,
            "parity": mask,
            "w1": w1eff,
            "w2": w2_bf,
            "b1": b1.reshape(C, 1),
            "b2": b2.reshape(C, 1),
        })

    nc = _get_nc()
    res = run_bass_kernel_spmd(nc, in_maps, list(range(N_CORES)),
                               trace=_trace, **(_trace_kwargs or {}))

    out = np.empty((E_TOTAL, C), dtype=np.float32)
    for i in range(N_CORES):
        out[i * EPC:(i + 1) * EPC] = res.results[i]["y"][:EPC]
    if _trace:
        return out, res
    return out


# revision 21
# speedup vs baseline: 42162.4918x; 1.0041x over previous
"""GNN message-passing layer on 8 TRN2 NeuronCores.

Math: y[e] = relu(concat(x[i[e]], x[i[e]]) @ W1 + b1) @ W2 + b2
         = relu(x[i[e]] @ (W1[:C]+W1[C:]) + b1) @ W2 + b2.
The MLP depends only on the source node, so compute z = MLP(x) once per
node (50k rows), then y = z[nbr_idx] is a pure gather (800k rows).

Sharding: edges are split evenly across the 8 cores; each core computes
the full z table locally (x + weights replicated; phase A is tiny) and
then gathers + writes its own edge shard. No collectives.

Phase B uses the GPSIMD dma_gather custom instruction. Its indices are
signed int16, so the bf16 z table is gathered at pair-row granularity
(row = 2 nodes = 512B, pair id < 25088 fits int16); a DVE predicated
copy then selects the right half per edge (mask = idx & 1) and upcasts
to f32. Edge->position packing is chosen so the per-tile y write is one
contiguous 8KB descriptor per partition.
"""

from contextlib import ExitStack

import ml_dtypes
import numpy as np

import concourse.bacc as bacc
import concourse.mybir as mybir
import concourse.tile as tile
from concourse import library_config
from concourse.bass_utils import run_bass_kernel_spmd
from concourse.masks import make_identity

N_CORES = 8
C = 128  # channels (C_IN == C_OUT)
N_NODES = 50000
E_TOTAL = 800000

ACH = 512  # phase-A compute chunk (max moving dim per matmul)
SCH = 2048  # phase-A DMA super-chunk (one x load + one z write)
NPAD = ((N_NODES + SCH - 1) // SCH) * SCH  # 51200
NCH = NPAD // ACH  # 100

EPC = E_TOTAL // N_CORES  # 100000 edges per core
NI = 2048  # edges per dma_gather tile
TBB = (EPC + NI - 1) // NI  # 49 gather tiles
EPC_PAD = TBB * NI  # 100352
KCH = NI // 128  # 16 gathered chunks per partition

F32 = mybir.dt.float32
BF16 = mybir.dt.bfloat16

# matmul input dtype for phase A
MM_DT = mybir.dt.bfloat16


import os
PHASES = os.environ.get("KPHASES", "AB")


def _build_nc():
    nc = bacc.Bacc("TRN2", target_bir_lowering=False, debug=False,
                   num_devices=N_CORES, dynamic_dma_scratch_size=131072)

    xT = nc.dram_tensor("xT", [C, NPAD], BF16, kind="ExternalInput")
    idx16 = nc.dram_tensor("idx16", [128, EPC_PAD // 16], mybir.dt.int16,
                           kind="ExternalInput")
    parity = nc.dram_tensor("parity", [128, EPC_PAD // 128], mybir.dt.uint8,
                            kind="ExternalInput")
    w1 = nc.dram_tensor("w1", [C, C], BF16, kind="ExternalInput")
    w2 = nc.dram_tensor("w2", [C, C], BF16, kind="ExternalInput")
    b1 = nc.dram_tensor("b1", [C, 1], F32, kind="ExternalInput")
    b2 = nc.dram_tensor("b2", [C, 1], F32, kind="ExternalInput")
    y = nc.dram_tensor("y", [EPC_PAD, C], F32, kind="ExternalOutput")
    zkind = "ExternalOutput" if PHASES == "A" else \
        ("ExternalInput" if PHASES == "B" else "Internal")
    z = nc.dram_tensor("z_table", [NPAD, C], BF16, kind=zkind)

    with tile.TileContext(nc) as tc, ExitStack() as ctx:
        const = ctx.enter_context(tc.tile_pool(name="const", bufs=1))
        xpool = ctx.enter_context(tc.tile_pool(name="xin", bufs=2))
        hpool = ctx.enter_context(tc.tile_pool(name="hbuf", bufs=3))
        zb_pool = ctx.enter_context(tc.tile_pool(name="zb", bufs=3))
        gpool = ctx.enter_context(tc.tile_pool(name="gbuf", bufs=4))
        spool = ctx.enter_context(tc.tile_pool(name="sel", bufs=3))
        psA = ctx.enter_context(tc.tile_pool(name="psA", bufs=2, space="PSUM"))
        psT = ctx.enter_context(tc.tile_pool(name="psT", bufs=2, space="PSUM"))

        w1t = const.tile([C, C], MM_DT)
        w2t = const.tile([C, C], MM_DT)
        b1t = const.tile([C, 1], F32)
        b2t = const.tile([C, 1], F32)
        ident = const.tile([128, 128], BF16)
        idxt = const.tile([128, EPC_PAD // 16], mybir.dt.int16)
        maskt = const.tile([128, EPC_PAD // 128], mybir.dt.uint8)
        nc.sync.dma_start(out=w1t[:], in_=w1[:])
        nc.sync.dma_start(out=w2t[:], in_=w2[:])
        nc.sync.dma_start(out=b1t[:], in_=b1[:])
        nc.sync.dma_start(out=b2t[:], in_=b2[:])
        nc.sync.dma_start(out=idxt[:], in_=idx16[:])
        nc.sync.dma_start(out=maskt[:], in_=parity[:])
        make_identity(nc, ident[:])

        # ---- Phase A (skipped when PHASES=="B"):
        # z[n] = relu(x[n] @ W1eff + b1) @ W2 + b2 per 512-node chunk in
        # transposed orientation, PE-transposed back in 4-interleaved column
        # groups. DMA granularity is a 2048-node super-chunk: one x load and
        # one z write each.
        SUB = SCH // ACH
        for ts in range(NPAD // SCH if "A" in PHASES else 0):
            xt = xpool.tile([C, SCH], MM_DT)
            nc.sync.dma_start(out=xt[:], in_=xT[:, ts * SCH:(ts + 1) * SCH])
            # zbuf[q, b, j, c] = z[ts*SCH + 512b + 4q + j, c]
            zbuf = zb_pool.tile([128, SUB, ACH // 128, C], BF16, tag="zbuf")
            for b in range(SUB):
                h_ps = psA.tile([C, ACH], F32, tag="h_ps")
                nc.tensor.matmul(h_ps[:], w1t[:], xt[:, b * ACH:(b + 1) * ACH],
                                 start=True, stop=True)
                h_sb = hpool.tile([C, ACH], MM_DT, tag="h_sb")
                nc.scalar.activation(h_sb[:], h_ps[:],
                                     mybir.ActivationFunctionType.Relu,
                                     bias=b1t[:, 0:1])

                z_ps = psA.tile([C, ACH], F32, tag="z_ps")
                nc.tensor.matmul(z_ps[:], w2t[:], h_sb[:], start=True,
                                 stop=True)
                zt_sb = hpool.tile([C, ACH], BF16, tag="zt_sb")
                nc.vector.tensor_tensor(out=zt_sb[:], in0=z_ps[:],
                                        in1=b2t[:, 0:1].to_broadcast([C, ACH]),
                                        op=mybir.AluOpType.add)

                # transpose col group j (cols j, j+4, ...) -> rows +4q+j
                tr_ps = psT.tile([128, ACH // 128, 128], BF16, tag="tr")
                for j in range(ACH // 128):
                    nc.tensor.transpose(tr_ps[:, j, :], zt_sb[:, j:ACH:4],
                                        ident[:])
                if b % 2 == 0:
                    nc.vector.tensor_copy(zbuf[:, b, :, :], tr_ps[:])
                else:
                    nc.scalar.copy(zbuf[:, b, :, :], tr_ps[:])
            n0 = ts * SCH
            nc.scalar.dma_start(
                out=z[n0:n0 + SCH, :].rearrange("(b q j) c -> q b j c",
                                                b=SUB, j=4),
                in_=zbuf[:])

        tc.strict_bb_all_engine_barrier()

        # ---- Phase B: dma_gather pair-rows + DVE half-select, write y shard.
        nc.gpsimd.load_library(library_config.mlp)
        zview = z[:].rearrange("(a two) c -> a (two c)", two=2)  # [NPAD/2,2C]
        for t in range(TBB if "B" in PHASES else 0):
            g = gpool.tile([128, KCH, 2 * C], BF16, tag="g")
            nc.gpsimd.dma_gather(
                out_ap=g[:], in_ap=zview,
                idxs_ap=idxt[:, t * (NI // 16):(t + 1) * (NI // 16)],
                num_idxs=NI, num_idxs_reg=NI, elem_size=2 * C,
                single_packet=False)
            even = g[:, :, 0:C]
            odd = g[:, :, C:2 * C]
            m = maskt[:, t * KCH:(t + 1) * KCH].to_broadcast([128, KCH, C])
            nc.vector.copy_predicated(out=even, mask=m, data=odd)
            sel = spool.tile([128, KCH, C], F32, tag="sel")
            nc.scalar.copy(sel[:], even)
            # position (p, tl) holds edge row p*KCH + tl of this tile;
            # alternate the two HWDGE rings for the big y writes
            weng = nc.sync if t % 2 == 0 else nc.scalar
            weng.dma_start(
                out=y[t * NI:(t + 1) * NI, :].rearrange(
                    "(p tl) c -> p tl c", tl=KCH),
                in_=sel[:])

    nc.compile()
    return nc


_NC_CACHE = None


def _get_nc():
    global _NC_CACHE
    if _NC_CACHE is None:
        _NC_CACHE = _build_nc()
    return _NC_CACHE


def _pack_indices(idx_pad):
    """idx_pad: int32 [EPC_PAD] -> (idx16 [128, EPC_PAD//16] int16,
    parity [128, EPC_PAD//128] bf16) in the position layout where edge row
    r (within a tile) sits at gather position i = (r%16)*128 + r//16."""
    pair = (idx_pad >> 1).astype(np.int16)
    par = (idx_pad & 1).astype(np.uint8)

    r = np.arange(NI)
    pos = (r % 16) * 128 + r // 16  # position of row r

    pair_t = pair.reshape(TBB, NI)
    pair_by_pos = np.empty((TBB, NI), dtype=np.int16)
    pair_by_pos[:, pos] = pair_t
    # wrap: position i at [i%16, i//16] per tile, tiles side by side
    idx16 = (pair_by_pos.reshape(TBB, NI // 16, 16)
             .transpose(2, 0, 1).reshape(16, TBB * (NI // 16)))
    idx16 = np.tile(np.ascontiguousarray(idx16), (8, 1))

    # mask[p, t*KCH + tl] = parity of edge row p*KCH + tl of tile t
    mask = (par.reshape(TBB, 128, KCH).transpose(1, 0, 2)
            .reshape(128, TBB * KCH))
    return idx16, np.ascontiguousarray(mask).astype(np.uint8)


def kernel(x, nbr_idx, W1, b1, W2, b2, _trace=False, _trace_kwargs=None):
    x = np.asarray(x, dtype=np.float32)
    nbr_idx_np = np.asarray(nbr_idx)
    W1 = np.asarray(W1, dtype=np.float32)
    W2 = np.asarray(W2, dtype=np.float32)
    b1 = np.asarray(b1, dtype=np.float32)
    b2 = np.asarray(b2, dtype=np.float32)

    w1eff = np.ascontiguousarray(W1[:C] + W1[C:]).astype(ml_dtypes.bfloat16)
    w2_bf = W2.astype(ml_dtypes.bfloat16)
    xT = np.zeros((C, NPAD), dtype=ml_dtypes.bfloat16)
    xT[:, :N_NODES] = x.T.astype(ml_dtypes.bfloat16)

    in_maps = []
    for i in range(N_CORES):
        idx_pad = np.zeros(EPC_PAD, dtype=np.int32)
        idx_pad[:EPC] = nbr_idx_np[i * EPC:(i + 1) * EPC].astype(np.int32)
        idx16, mask = _pack_indices(idx_pad)
        in_maps.append({
            "xT": xT,
            "idx16": idx16,
            "parity": mask,
            "w1": w1eff,
            "w2": w2_bf,
            "b1": b1.reshape(C, 1),
            "b2": b2.reshape(C, 1),
        })

    nc = _get_nc()
    res = run_bass_kernel_spmd(nc, in_maps, list(range(N_CORES)),
                               trace=_trace, **(_trace_kwargs or {}))

    out = np.empty((E_TOTAL, C), dtype=np.float32)
    for i in range(N_CORES):
        out[i * EPC:(i + 1) * EPC] = res.results[i]["y"][:EPC]
    if _trace:
        return out, res
    return out
